# revision 9
# baseline (speedup 1.0000x reference)
"""AutoRegRNN (LSTM warmup scan + autoregressive forecast) on 8 trn2 NeuronCores.

Sharding: data-parallel over batch (B=512 -> 64 per core); LSTM weights
replicated; each core runs the full sequential scan on its batch shard.

Per-core layout (fp32 everywhere):
  - Gates z_t [64, 2048] batch-major in PSUM, accumulated as
    z = xW_ext (K=F+1 via ones-row trick, folds bias b) + sum_k hT_k @ U_k.
  - Stationary operands are the transposed state chunks hT [128, 64] and the
    transposed input xT_t [128, 64] (rows 0..63 = x_t.T, row 64 = ones,
    rest zero), produced on-PE with tensor.transpose.
  - Gate nonlinearities on ACT reading PSUM directly; cell/hidden updates on
    DVE; h re-transposed on PE each step for the next step's stationary.
  - Forecast feeds the prediction back through the same machinery with
    pT [128, 64] as stationary; bd folded into the pred matmul with a
    ones-chunk as a 5th contraction chunk of Wd.
"""

import numpy as np

B, T, F, UNITS, OUT, STEPS = 512, 1024, 64, 512, 64, 128
NCORES = 8
BS = B // NCORES  # 64 batch rows per core
G = 4 * UNITS  # 2048
KO = UNITS // 128  # 4 hidden chunks
BODY = 32  # warmup steps per For_i iteration (two 16-step DMA half-chunks)
HALF = 16


def _build(nc, T=T, STEPS=STEPS):
    import concourse.bass as bass
    import concourse.mybir as mybir
    import concourse.tile as tile
    from concourse.bass import ds
    from concourse.masks import make_identity

    f32 = mybir.dt.float32
    AF = mybir.ActivationFunctionType

    x_d = nc.dram_tensor("x", [BS, T, F], f32, kind="ExternalInput")
    w_d = nc.dram_tensor("W", [F, G], f32, kind="ExternalInput")
    u_d = nc.dram_tensor("U", [UNITS, G], f32, kind="ExternalInput")
    b_d = nc.dram_tensor("b", [G], f32, kind="ExternalInput")
    wd_d = nc.dram_tensor("Wd", [UNITS, OUT], f32, kind="ExternalInput")
    bd_d = nc.dram_tensor("bd", [OUT], f32, kind="ExternalInput")
    out_d = nc.dram_tensor("out", [BS, STEPS, OUT], f32, kind="ExternalOutput")

    n_bodies = T // BODY
    assert T % BODY == 0 and n_bodies >= 1

    with tile.TileContext(nc) as tc:
        with (
            tc.tile_pool(name="persist", bufs=1) as pp,
            tc.tile_pool(name="work", bufs=2) as wp,
            tc.tile_pool(name="zps", bufs=1, space="PSUM") as zpool,
            tc.tile_pool(name="tps", bufs=2, space="PSUM") as tpool,
        ):
            # ---- persistent tensors ----
            u_sb = pp.tile([128, KO, G], f32)  # U: [p, ko, n], row ko*128+p
            wx_sb = pp.tile([128, G], f32)  # rows 0..63 W, row 64 = b, rest 0
            wd_sb = pp.tile([128, KO + 1, OUT], f32)  # chunk KO: row0 = bd
            ones_c = pp.tile([128, OUT], f32)  # row 0 = 1, rest 0
            ident = pp.tile([64, 64], f32)
            xsb = pp.tile([BS, BODY, F], f32)  # x staging, 2 half-chunks
            xT = pp.tile([128, BODY, BS], f32)  # row 64 = ones, 65.. = 0
            pT = [pp.tile([128, BS], f32, tag=f"pT{i}", name=f"pT{i}") for i in range(2)]
            hT = [pp.tile([128, KO, BS], f32, tag=f"hT{i}", name=f"hT{i}") for i in range(2)]
            cst = [pp.tile([BS, UNITS], f32, tag=f"c{i}", name=f"c{i}") for i in range(2)]
            preds = pp.tile([BS, STEPS, OUT], f32)

            # ---- init ----
            make_identity(nc, ident)
            nc.gpsimd.memset(wx_sb, 0.0)
            nc.sync.dma_start(wx_sb[:F, :], w_d[:, :])
            nc.sync.dma_start(wx_sb[F : F + 1, :], b_d[None, :])
            nc.sync.dma_start(
                u_sb, u_d.rearrange("(ko p) n -> p ko n", p=128)
            )
            nc.gpsimd.memset(wd_sb, 0.0)
            nc.sync.dma_start(
                wd_sb[:, :KO, :], wd_d.rearrange("(ko p) n -> p ko n", p=128)
            )
            nc.sync.dma_start(wd_sb[0:1, KO, :], bd_d[None, :])
            nc.gpsimd.memset(ones_c, 0.0)
            nc.gpsimd.memset(ones_c[0:1, :], 1.0)
            nc.gpsimd.memset(xT, 0.0)
            nc.gpsimd.memset(xT[F : F + 1, :, :], 1.0)
            nc.gpsimd.memset(pT[0], 0.0)
            nc.gpsimd.memset(pT[0][F : F + 1, :], 1.0)
            nc.gpsimd.memset(pT[1], 0.0)
            nc.gpsimd.memset(pT[1][F : F + 1, :], 1.0)
            nc.gpsimd.memset(hT[0], 0.0)
            nc.gpsimd.memset(cst[0], 0.0)
            nc.gpsimd.memset(hT[1], 0.0)
            nc.gpsimd.memset(cst[1], 0.0)

            # first x chunks + transpose of step 0
            nc.sync.dma_start(xsb[:, :HALF, :], x_d[:, :HALF, :])
            if T > HALF:
                nc.sync.dma_start(xsb[:, HALF:, :], x_d[:, HALF:BODY, :])
            tps0 = tpool.tile([64, BS], f32, tag="xtp")
            nc.tensor.transpose(tps0, xsb[:, 0, :], ident)
            nc.vector.tensor_copy(out=xT[:F, 0, :], in_=tps0)

            state = [0]  # ping-pong index, python-side

            def emit_step(stat_ap, xpose_in_ap, xpose_slot):
                """One LSTM cell step.

                stat_ap: [128, BS] stationary for the input-side matmul
                    (xT slot or pT).
                xpose_in_ap/xpose_slot: batch-major [64, F] x to transpose
                    into xT[:, xpose_slot, :] for an upcoming step (or None).
                Returns the h tile (batch-major) of this step.
                """
                cur = state[0]
                nxt = 1 - cur
                state[0] = nxt

                zps = zpool.tile([BS, G], f32, tag="z")
                # input-side matmuls (start accumulation groups)
                for n in range(4):
                    nc.tensor.matmul(
                        zps[:, n * 512 : (n + 1) * 512],
                        stat_ap,
                        wx_sb[:, n * 512 : (n + 1) * 512],
                        start=True,
                        stop=False,
                    )
                # recurrent matmuls
                for ko in range(KO):
                    for n in range(4):
                        nc.tensor.matmul(
                            zps[:, n * 512 : (n + 1) * 512],
                            hT[cur][:, ko, :],
                            u_sb[:, ko, n * 512 : (n + 1) * 512],
                            start=False,
                            stop=(ko == KO - 1),
                        )
                # fill PE stall with next step's x transpose
                if xpose_in_ap is not None:
                    tps = tpool.tile([64, BS], f32, tag="xtp")
                    nc.tensor.transpose(tps, xpose_in_ap, ident)
                    nc.vector.tensor_copy(out=xT[:F, xpose_slot, :], in_=tps)

                # gates
                g_if = wp.tile([BS, 2 * UNITS], f32, tag="gif")
                g_g = wp.tile([BS, UNITS], f32, tag="gg")
                g_o = wp.tile([BS, UNITS], f32, tag="go")
                nc.scalar.activation(g_if, zps[:, : 2 * UNITS], AF.Sigmoid)
                nc.scalar.activation(g_g, zps[:, 2 * UNITS : 3 * UNITS], AF.Tanh)
                nc.scalar.activation(g_o, zps[:, 3 * UNITS :], AF.Sigmoid)
                # state update
                ig = wp.tile([BS, UNITS], f32, tag="ig")
                nc.vector.tensor_mul(out=ig, in0=g_if[:, :UNITS], in1=g_g)
                nc.vector.tensor_mul(out=cst[nxt], in0=g_if[:, UNITS:], in1=cst[cur])
                nc.vector.tensor_add(out=cst[nxt], in0=cst[nxt], in1=ig)
                tc_t = wp.tile([BS, UNITS], f32, tag="tc")
                nc.scalar.activation(tc_t, cst[nxt], AF.Tanh)
                h = wp.tile([BS, UNITS], f32, tag="h")
                nc.vector.tensor_mul(out=h, in0=g_o, in1=tc_t)
                # transpose h for next step's stationary
                hps = tpool.tile([128, KO, BS], f32, tag="htp")
                for ko in range(KO):
                    nc.tensor.transpose(
                        hps[:, ko, :], h[:, ko * 128 : (ko + 1) * 128], ident
                    )
                nc.vector.tensor_copy(out=hT[nxt], in_=hps)
                return h

            def warm_body(m, dyn):
                # Steps m*BODY .. m*BODY+31 from static slots; refill each
                # 16-slot half right after its last reader so only the DMA
                # *source* offset is dynamic (ldweights can't take reg APs).
                for k in range(BODY):
                    last_step = (not dyn) and k == BODY - 1
                    if last_step:
                        xp_in, xp_slot = None, None
                    else:
                        xp_slot = (k + 1) % BODY
                        xp_in = xsb[:, xp_slot, :]
                    emit_step(xT[:, k, :], xp_in, xp_slot)
                    if dyn and k == HALF - 1:
                        nc.sync.dma_start(
                            xsb[:, :HALF, :], x_d[:, ds((m + 1) * BODY, HALF), :]
                        )
                    elif dyn and k == BODY - 1:
                        nc.sync.dma_start(
                            xsb[:, HALF:, :],
                            x_d[:, ds((m + 1) * BODY + HALF, HALF), :],
                        )

            if n_bodies > 1:
                with tc.For_i(0, n_bodies - 1, 1) as jj:
                    warm_body(jj, dyn=True)
            warm_body(n_bodies - 1, dyn=False)

            # ---- forecast ----
            def emit_pred(s):
                """pred = h @ Wd + bd from hT[state], into preds[:, s, :];
                transpose into pT for the next cell step."""
                cur = state[0]
                pps = tpool.tile([BS, OUT], f32, tag="xtp")
                for ko in range(KO):
                    nc.tensor.matmul(
                        pps,
                        hT[cur][:, ko, :],
                        wd_sb[:, ko, :],
                        start=(ko == 0),
                        stop=False,
                    )
                nc.tensor.matmul(pps, ones_c, wd_sb[:, KO, :], start=False, stop=True)
                nc.vector.tensor_copy(out=preds[:, s, :], in_=pps)
                if s < STEPS - 1:
                    ptp = tpool.tile([64, BS], f32, tag="xtp")
                    nc.tensor.transpose(ptp, preds[:, s, :], ident)
                    nc.vector.tensor_copy(out=pT[s % 2][:F, :], in_=ptp)

            emit_pred(0)
            for s in range(1, STEPS):
                emit_step(pT[(s - 1) % 2], None, None)
                emit_pred(s)

            nc.sync.dma_start(out_d[:, :, :], preds)

    nc.finalize()
    return nc


_cache = {}


def _get_nc(Tv, Sv):
    key = (Tv, Sv)
    if key not in _cache:
        from concourse import bacc

        nc = bacc.Bacc()
        _cache[key] = _build(nc, Tv, Sv)
    return _cache[key]


def kernel(x, W, U, b, Wd, bd, forecast_steps, _trace=False):
    x = np.ascontiguousarray(x, dtype=np.float32)
    Tv = x.shape[1]
    Sv = int(forecast_steps)
    assert x.shape[0] == B and x.shape[2] == F

    global T, STEPS
    T, STEPS = Tv, Sv  # allow reduced configs in testing
    nc = _get_nc(Tv, Sv)

    shared = {
        "W": np.ascontiguousarray(W, dtype=np.float32),
        "U": np.ascontiguousarray(U, dtype=np.float32),
        "b": np.ascontiguousarray(b, dtype=np.float32),
        "Wd": np.ascontiguousarray(Wd, dtype=np.float32),
        "bd": np.ascontiguousarray(bd, dtype=np.float32),
    }
    in_maps = [
        dict(shared, x=np.ascontiguousarray(x[c * BS : (c + 1) * BS]))
        for c in range(NCORES)
    ]
    from concourse.bass_utils import run_bass_kernel_spmd

    res = run_bass_kernel_spmd(
        nc, in_maps, core_ids=list(range(NCORES)), trace=_trace
    )
    out = np.concatenate([r["out"] for r in res.results], axis=0)
    if _trace:
        return out, res
    return out


# revision 18
# speedup vs baseline: 174.1317x; 174.1317x over previous
"""AutoRegRNN (LSTM warmup scan + autoregressive forecast) on 8 trn2 NeuronCores.

Sharding: data-parallel over batch (B=512 -> 64 per core); LSTM weights
replicated; each core runs the full sequential scan on its batch shard.

Per-core layout (fp32 everywhere):
  - Gates z_t [64, 2048] batch-major in PSUM, accumulated as
    z = xW_ext (K=F+1 via ones-row trick, folds bias b) + sum_k hT_k @ U_k.
  - Stationary operands are the transposed state chunks hT [128, 64] and the
    transposed input xT_t [128, 64] (rows 0..63 = x_t.T, row 64 = ones,
    rest zero), produced on-PE with tensor.transpose.
  - Gate nonlinearities on ACT reading PSUM directly; cell/hidden updates on
    DVE; h re-transposed on PE each step for the next step's stationary.
  - Forecast feeds the prediction back through the same machinery with
    pT [128, 64] as stationary; bd folded into the pred matmul with a
    ones-chunk as a 5th contraction chunk of Wd.
"""

import numpy as np

B, T, F, UNITS, OUT, STEPS = 512, 1024, 64, 512, 64, 128
NCORES = 8
BS = B // NCORES  # 64 batch rows per core
G = 4 * UNITS  # 2048
KO = UNITS // 128  # 4 hidden chunks
BODY = 32  # warmup steps per For_i iteration (two 16-step DMA half-chunks)
HALF = 16


def _build(nc, T=T, STEPS=STEPS, rounds=1):
    import concourse.bass as bass
    import concourse.mybir as mybir
    import concourse.tile as tile
    from concourse.bass import ds
    from concourse.masks import make_identity

    f32 = mybir.dt.float32
    f32r = mybir.dt.float32r  # reduced-precision fp32 matmul: 1 cyc/row vs 4
    AF = mybir.ActivationFunctionType

    x_d = nc.dram_tensor("x", [BS, T, F], f32, kind="ExternalInput")
    w_d = nc.dram_tensor("W", [F, G], f32, kind="ExternalInput")
    u_d = nc.dram_tensor("U", [UNITS, G], f32, kind="ExternalInput")
    b_d = nc.dram_tensor("b", [G], f32, kind="ExternalInput")
    wd_d = nc.dram_tensor("Wd", [UNITS, OUT], f32, kind="ExternalInput")
    bd_d = nc.dram_tensor("bd", [OUT], f32, kind="ExternalInput")
    out_d = nc.dram_tensor("out", [BS, STEPS, OUT], f32, kind="ExternalOutput")

    n_bodies = T // BODY
    assert T % BODY == 0 and n_bodies >= 1

    with tile.TileContext(nc) as tc:
        with (
            tc.tile_pool(name="persist", bufs=1) as pp,
            tc.tile_pool(name="work", bufs=2) as wp,
            tc.tile_pool(name="zps", bufs=1, space="PSUM") as zpool,
            tc.tile_pool(name="tps", bufs=2, space="PSUM") as tpool,
        ):
            # ---- persistent tensors (matmul operands in f32r) ----
            u_sb = pp.tile([128, KO, G], f32r)  # U: [p, ko, n], row ko*128+p
            wx_sb = pp.tile([128, G], f32r)  # rows 0..63 W, row 64 = b, rest 0
            wd_sb = pp.tile([128, KO + 1, OUT], f32r)  # chunk KO: row0 = bd
            ones_c = pp.tile([128, OUT], f32r)  # row 0 = 1, rest 0
            ident = pp.tile([64, 64], f32)
            xsb = pp.tile([BS, BODY, F], f32)  # x staging, 2 half-chunks
            xT = pp.tile([128, BODY, BS], f32r)  # row 64 = ones, 65.. = 0
            pT = [pp.tile([128, BS], f32r, tag=f"pT{i}", name=f"pT{i}") for i in range(2)]
            hT = [pp.tile([128, KO, BS], f32r, tag=f"hT{i}", name=f"hT{i}") for i in range(2)]
            cst = [pp.tile([BS, UNITS], f32, tag=f"c{i}", name=f"c{i}") for i in range(2)]
            preds = pp.tile([BS, STEPS, OUT], f32)

            # ---- init (DMA to fp32 staging, DVE round-copy into f32r) ----
            make_identity(nc, ident)
            for ko in range(KO):
                stg = wp.tile([128, G], f32, tag="stage", name="stg_u")
                nc.sync.dma_start(
                    stg, u_d.rearrange("(ko p) n -> p ko n", p=128)[:, ko, :]
                )
                nc.vector.tensor_copy(out=u_sb[:, ko, :], in_=stg)
            stg = wp.tile([128, G], f32, tag="stage", name="stg_wx")
            nc.gpsimd.memset(stg, 0.0)
            nc.sync.dma_start(stg[:F, :], w_d[:, :])
            nc.sync.dma_start(stg[F : F + 1, :], b_d[None, :])
            nc.vector.tensor_copy(out=wx_sb, in_=stg)
            stg = wp.tile([128, (KO + 1) * OUT], f32, tag="stage", name="stg_wd")
            nc.gpsimd.memset(stg, 0.0)
            nc.sync.dma_start(
                stg[:, : KO * OUT].rearrange("p (ko n) -> p ko n", ko=KO),
                wd_d.rearrange("(ko p) n -> p ko n", p=128),
            )
            nc.sync.dma_start(stg[0:1, KO * OUT :], bd_d[None, :])
            nc.vector.tensor_copy(
                out=wd_sb, in_=stg.rearrange("p (ko n) -> p ko n", ko=KO + 1)
            )
            stg = wp.tile([128, OUT], f32, tag="stage", name="stg_ones")
            nc.gpsimd.memset(stg, 0.0)
            nc.gpsimd.memset(stg[0:1, :], 1.0)
            nc.vector.tensor_copy(out=ones_c, in_=stg)
            stg = wp.tile([128, BODY * BS], f32, tag="stage", name="stg_xt")
            nc.gpsimd.memset(stg, 0.0)
            nc.gpsimd.memset(stg[F : F + 1, :], 1.0)
            nc.vector.tensor_copy(
                out=xT, in_=stg.rearrange("p (t b) -> p t b", t=BODY)
            )
            stg = wp.tile([128, BS], f32, tag="stage", name="stg_pt")
            nc.gpsimd.memset(stg, 0.0)
            nc.gpsimd.memset(stg[F : F + 1, :], 1.0)
            nc.vector.tensor_copy(out=pT[0], in_=stg)
            nc.vector.tensor_copy(out=pT[1], in_=stg)
            zro = pp.tile([128, KO, BS], f32)
            nc.gpsimd.memset(zro, 0.0)

            state = [0]  # ping-pong index, python-side

            def round_prologue():
                nc.vector.tensor_copy(out=hT[0], in_=zro)
                nc.vector.tensor_copy(out=hT[1], in_=zro)
                nc.gpsimd.memset(cst[0], 0.0)
                nc.gpsimd.memset(cst[1], 0.0)
                # first x chunks + transpose of step 0
                nc.sync.dma_start(xsb[:, :HALF, :], x_d[:, :HALF, :])
                if T > HALF:
                    nc.sync.dma_start(xsb[:, HALF:, :], x_d[:, HALF:BODY, :])
                tps0 = tpool.tile([64, BS], f32, tag="xtp")
                nc.tensor.transpose(tps0, xsb[:, 0, :], ident)
                nc.vector.tensor_copy(out=xT[:F, 0, :], in_=tps0)

            def emit_step(stat_ap, xpose_in_ap, xpose_slot):
                """One LSTM cell step.

                stat_ap: [128, BS] stationary for the input-side matmul
                    (xT slot or pT).
                xpose_in_ap/xpose_slot: batch-major [64, F] x to transpose
                    into xT[:, xpose_slot, :] for an upcoming step (or None).
                Returns the h tile (batch-major) of this step.
                """
                cur = state[0]
                nxt = 1 - cur
                state[0] = nxt

                zps = zpool.tile([BS, G], f32, tag="z")

                def gate_mms(n):
                    # one gate's accumulation group: xW (starts) + 4 U chunks
                    nc.tensor.matmul(
                        zps[:, n * 512 : (n + 1) * 512],
                        stat_ap,
                        wx_sb[:, n * 512 : (n + 1) * 512],
                        start=True,
                        stop=False,
                    )
                    for ko in range(KO):
                        nc.tensor.matmul(
                            zps[:, n * 512 : (n + 1) * 512],
                            hT[cur][:, ko, :],
                            u_sb[:, ko, n * 512 : (n + 1) * 512],
                            start=False,
                            stop=(ko == KO - 1),
                        )

                # gate order i, g, f, o: each gate's ACT overlaps the next
                # gate's matmuls; o last so h is ready soonest after the MMs.
                g_i = wp.tile([BS, UNITS], f32, tag="gi")
                g_g = wp.tile([BS, UNITS], f32, tag="gg")
                g_f = wp.tile([BS, UNITS], f32, tag="gf")
                g_o = wp.tile([BS, UNITS], f32, tag="go")
                ig = wp.tile([BS, UNITS], f32, tag="ig")
                tc_t = wp.tile([BS, UNITS], f32, tag="tc")
                h = wp.tile([BS, UNITS], f32, tag="h")

                gate_mms(0)
                nc.scalar.activation(g_i, zps[:, :UNITS], AF.Sigmoid)
                gate_mms(2)
                nc.scalar.activation(g_g, zps[:, 2 * UNITS : 3 * UNITS], AF.Tanh)
                nc.vector.tensor_mul(out=ig, in0=g_i, in1=g_g)
                gate_mms(1)
                nc.scalar.activation(g_f, zps[:, UNITS : 2 * UNITS], AF.Sigmoid)
                nc.vector.tensor_mul(out=cst[nxt], in0=g_f, in1=cst[cur])
                nc.vector.tensor_add(out=cst[nxt], in0=cst[nxt], in1=ig)
                nc.scalar.activation(tc_t, cst[nxt], AF.Tanh)
                gate_mms(3)
                nc.scalar.activation(g_o, zps[:, 3 * UNITS :], AF.Sigmoid)
                # fill PE stall with next step's x transpose
                if xpose_in_ap is not None:
                    tps = tpool.tile([64, BS], f32, tag="xtp")
                    nc.tensor.transpose(tps, xpose_in_ap, ident)
                    nc.vector.tensor_copy(out=xT[:F, xpose_slot, :], in_=tps)
                nc.vector.tensor_mul(out=h, in0=g_o, in1=tc_t)
                # transpose h for next step's stationary
                hps = tpool.tile([128, KO, BS], f32, tag="htp")
                for ko in range(KO):
                    nc.tensor.transpose(
                        hps[:, ko, :], h[:, ko * 128 : (ko + 1) * 128], ident
                    )
                nc.vector.tensor_copy(out=hT[nxt], in_=hps)
                return h

            def warm_body(m, dyn):
                # Steps m*BODY .. m*BODY+31 from static slots; refill each
                # 16-slot half right after its last reader so only the DMA
                # *source* offset is dynamic (ldweights can't take reg APs).
                for k in range(BODY):
                    last_step = (not dyn) and k == BODY - 1
                    if last_step:
                        xp_in, xp_slot = None, None
                    else:
                        xp_slot = (k + 1) % BODY
                        xp_in = xsb[:, xp_slot, :]
                    emit_step(xT[:, k, :], xp_in, xp_slot)
                    if dyn and k == HALF - 1:
                        nc.sync.dma_start(
                            xsb[:, :HALF, :], x_d[:, ds((m + 1) * BODY, HALF), :]
                        )
                    elif dyn and k == BODY - 1:
                        nc.sync.dma_start(
                            xsb[:, HALF:, :],
                            x_d[:, ds((m + 1) * BODY + HALF, HALF), :],
                        )

            def full_round():
                round_prologue()
                if n_bodies > 1:
                    with tc.For_i(0, n_bodies - 1, 1) as jj:
                        warm_body(jj, dyn=True)
                warm_body(n_bodies - 1, dyn=False)
                forecast()

            # ---- forecast ----
            def emit_pred(s):
                """pred = h @ Wd + bd from hT[state], into preds[:, s, :];
                transpose into pT for the next cell step."""
                cur = state[0]
                pps = tpool.tile([BS, OUT], f32, tag="xtp")
                for ko in range(KO):
                    nc.tensor.matmul(
                        pps,
                        hT[cur][:, ko, :],
                        wd_sb[:, ko, :],
                        start=(ko == 0),
                        stop=False,
                    )
                nc.tensor.matmul(pps, ones_c, wd_sb[:, KO, :], start=False, stop=True)
                nc.vector.tensor_copy(out=preds[:, s, :], in_=pps)
                if s < STEPS - 1:
                    ptp = tpool.tile([64, BS], f32, tag="xtp")
                    nc.tensor.transpose(ptp, preds[:, s, :], ident)
                    nc.vector.tensor_copy(out=pT[s % 2][:F, :], in_=ptp)

            def forecast():
                emit_pred(0)
                for s in range(1, STEPS):
                    emit_step(pT[(s - 1) % 2], None, None)
                    emit_pred(s)

            if rounds > 1:
                with tc.For_i(0, rounds, 1) as _r:
                    full_round()
            else:
                full_round()

            nc.sync.dma_start(out_d[:, :, :], preds)

    nc.finalize()
    return nc


_cache = {}


def _get_nc(Tv, Sv, rounds=1):
    key = (Tv, Sv, rounds)
    if key not in _cache:
        from concourse import bacc

        nc = bacc.Bacc()
        _cache[key] = _build(nc, Tv, Sv, rounds)
    return _cache[key]


def kernel(x, W, U, b, Wd, bd, forecast_steps, _trace=False, _rounds=1):
    x = np.ascontiguousarray(x, dtype=np.float32)
    Tv = x.shape[1]
    Sv = int(forecast_steps)
    assert x.shape[0] == B and x.shape[2] == F

    global T, STEPS
    T, STEPS = Tv, Sv  # allow reduced configs in testing
    nc = _get_nc(Tv, Sv, _rounds)

    shared = {
        "W": np.ascontiguousarray(W, dtype=np.float32),
        "U": np.ascontiguousarray(U, dtype=np.float32),
        "b": np.ascontiguousarray(b, dtype=np.float32),
        "Wd": np.ascontiguousarray(Wd, dtype=np.float32),
        "bd": np.ascontiguousarray(bd, dtype=np.float32),
    }
    in_maps = [
        dict(shared, x=np.ascontiguousarray(x[c * BS : (c + 1) * BS]))
        for c in range(NCORES)
    ]
    from concourse.bass_utils import run_bass_kernel_spmd

    res = run_bass_kernel_spmd(
        nc, in_maps, core_ids=list(range(NCORES)), trace=_trace
    )
    out = np.concatenate([r["out"] for r in res.results], axis=0)
    if _trace:
        return out, res
    return out


# revision 21
# speedup vs baseline: 248.7840x; 1.4287x over previous
"""AutoRegRNN (LSTM warmup scan + autoregressive forecast) on 8 trn2 NeuronCores.

Sharding: data-parallel over batch (B=512 -> 64 per core); LSTM weights
replicated; each core runs the full sequential scan on its batch shard.

Per-core layout (fp32 everywhere):
  - Gates z_t [64, 2048] batch-major in PSUM, accumulated as
    z = xW_ext (K=F+1 via ones-row trick, folds bias b) + sum_k hT_k @ U_k.
  - Stationary operands are the transposed state chunks hT [128, 64] and the
    transposed input xT_t [128, 64] (rows 0..63 = x_t.T, row 64 = ones,
    rest zero), produced on-PE with tensor.transpose.
  - Gate nonlinearities on ACT reading PSUM directly; cell/hidden updates on
    DVE; h re-transposed on PE each step for the next step's stationary.
  - Forecast feeds the prediction back through the same machinery with
    pT [128, 64] as stationary; bd folded into the pred matmul with a
    ones-chunk as a 5th contraction chunk of Wd.
"""

import numpy as np

B, T, F, UNITS, OUT, STEPS = 512, 1024, 64, 512, 64, 128
NCORES = 8
BS = B // NCORES  # 64 batch rows per core
G = 4 * UNITS  # 2048
KO = UNITS // 128  # 4 hidden chunks
BODY = 32  # warmup steps per For_i iteration (two 16-step DMA half-chunks)
HALF = 16


def _build(nc, T=T, STEPS=STEPS, rounds=1):
    import concourse.bass as bass
    import concourse.mybir as mybir
    import concourse.tile as tile
    from concourse.bass import ds
    from concourse.masks import make_identity

    f32 = mybir.dt.float32
    f32r = mybir.dt.float32r  # reduced-precision fp32 matmul: 1 cyc/row vs 4
    AF = mybir.ActivationFunctionType

    x_d = nc.dram_tensor("x", [BS, T, F], f32, kind="ExternalInput")
    w_d = nc.dram_tensor("W", [F, G], f32, kind="ExternalInput")
    u_d = nc.dram_tensor("U", [UNITS, G], f32, kind="ExternalInput")
    b_d = nc.dram_tensor("b", [G], f32, kind="ExternalInput")
    wd_d = nc.dram_tensor("Wd", [UNITS, OUT], f32, kind="ExternalInput")
    bd_d = nc.dram_tensor("bd", [OUT], f32, kind="ExternalInput")
    out_d = nc.dram_tensor("out", [BS, STEPS, OUT], f32, kind="ExternalOutput")

    n_bodies = T // BODY
    assert T % BODY == 0 and n_bodies >= 1

    with tile.TileContext(nc) as tc:
        with (
            tc.tile_pool(name="persist", bufs=1) as pp,
            tc.tile_pool(name="work", bufs=2) as wp,
            tc.tile_pool(name="zps", bufs=5, space="PSUM") as zpool,
            tc.tile_pool(name="tps", bufs=2, space="PSUM") as tpool,
            tc.tile_pool(name="xps", bufs=1, space="PSUM") as xpool,
        ):
            # ---- persistent tensors (matmul operands in f32r) ----
            u_sb = pp.tile([128, KO, G], f32r)  # U: [p, ko, n], row ko*128+p
            wx_sb = pp.tile([128, G], f32r)  # rows 0..63 W, row 64 = b, rest 0
            wd_sb = pp.tile([128, KO + 1, OUT], f32r)  # chunk KO: row0 = bd
            ones_c = pp.tile([128, OUT], f32r)  # row 0 = 1, rest 0
            ident = pp.tile([64, 64], f32)
            xsb = pp.tile([BS, BODY, F], f32)  # x staging, 2 half-chunks
            xT = pp.tile([128, BODY, BS], f32r)  # row 64 = ones, 65.. = 0
            pT = [pp.tile([128, BS], f32r, tag=f"pT{i}", name=f"pT{i}") for i in range(2)]
            hT = [pp.tile([128, KO, BS], f32r, tag=f"hT{i}", name=f"hT{i}") for i in range(2)]
            cst = [pp.tile([BS, UNITS], f32, tag=f"c{i}", name=f"c{i}") for i in range(2)]
            preds = pp.tile([BS, STEPS, OUT], f32)

            # ---- init (DMA to fp32 staging, DVE round-copy into f32r) ----
            make_identity(nc, ident)
            for ko in range(KO):
                stg = wp.tile([128, G], f32, tag="stage", name="stg_u")
                nc.sync.dma_start(
                    stg, u_d.rearrange("(ko p) n -> p ko n", p=128)[:, ko, :]
                )
                nc.vector.tensor_copy(out=u_sb[:, ko, :], in_=stg)
            stg = wp.tile([128, G], f32, tag="stage", name="stg_wx")
            nc.gpsimd.memset(stg, 0.0)
            nc.sync.dma_start(stg[:F, :], w_d[:, :])
            nc.sync.dma_start(stg[F : F + 1, :], b_d[None, :])
            nc.vector.tensor_copy(out=wx_sb, in_=stg)
            stg = wp.tile([128, (KO + 1) * OUT], f32, tag="stage", name="stg_wd")
            nc.gpsimd.memset(stg, 0.0)
            nc.sync.dma_start(
                stg[:, : KO * OUT].rearrange("p (ko n) -> p ko n", ko=KO),
                wd_d.rearrange("(ko p) n -> p ko n", p=128),
            )
            nc.sync.dma_start(stg[0:1, KO * OUT :], bd_d[None, :])
            nc.vector.tensor_copy(
                out=wd_sb, in_=stg.rearrange("p (ko n) -> p ko n", ko=KO + 1)
            )
            stg = wp.tile([128, OUT], f32, tag="stage", name="stg_ones")
            nc.gpsimd.memset(stg, 0.0)
            nc.gpsimd.memset(stg[0:1, :], 1.0)
            nc.vector.tensor_copy(out=ones_c, in_=stg)
            stg = wp.tile([128, BODY * BS], f32, tag="stage", name="stg_xt")
            nc.gpsimd.memset(stg, 0.0)
            nc.gpsimd.memset(stg[F : F + 1, :], 1.0)
            nc.vector.tensor_copy(
                out=xT, in_=stg.rearrange("p (t b) -> p t b", t=BODY)
            )
            stg = wp.tile([128, BS], f32, tag="stage", name="stg_pt")
            nc.gpsimd.memset(stg, 0.0)
            nc.gpsimd.memset(stg[F : F + 1, :], 1.0)
            nc.vector.tensor_copy(out=pT[0], in_=stg)
            nc.vector.tensor_copy(out=pT[1], in_=stg)
            zro = pp.tile([128, KO, BS], f32)
            nc.gpsimd.memset(zro, 0.0)

            state = [0]  # ping-pong index, python-side

            def round_prologue():
                nc.vector.tensor_copy(out=hT[0], in_=zro)
                nc.vector.tensor_copy(out=hT[1], in_=zro)
                nc.gpsimd.memset(cst[0], 0.0)
                nc.gpsimd.memset(cst[1], 0.0)
                # first x chunks + transpose of step 0
                nc.sync.dma_start(xsb[:, :HALF, :], x_d[:, :HALF, :])
                if T > HALF:
                    nc.sync.dma_start(xsb[:, HALF:, :], x_d[:, HALF:BODY, :])
                tps0 = xpool.tile([64, BS], f32, tag="xtp")
                nc.tensor.transpose(tps0, xsb[:, 0, :], ident)
                nc.vector.tensor_copy(out=xT[:F, 0, :], in_=tps0)

            def emit_step(stat_ap, xpose_in_ap, xpose_slot):
                """One LSTM cell step.

                stat_ap: [128, BS] stationary for the input-side matmul
                    (xT slot or pT).
                xpose_in_ap/xpose_slot: batch-major [64, F] x to transpose
                    into xT[:, xpose_slot, :] for an upcoming step (or None).
                Returns the h tile (batch-major) of this step.
                """
                cur = state[0]
                nxt = 1 - cur
                state[0] = nxt

                def gate_mms(n):
                    # one gate's accumulation group: xW (starts) + 4 U chunks
                    zp = zpool.tile([BS, UNITS], f32, tag="z", name=f"z{n}")
                    nc.tensor.matmul(
                        zp,
                        stat_ap,
                        wx_sb[:, n * 512 : (n + 1) * 512],
                        start=True,
                        stop=False,
                    )
                    for ko in range(KO):
                        nc.tensor.matmul(
                            zp,
                            hT[cur][:, ko, :],
                            u_sb[:, ko, n * 512 : (n + 1) * 512],
                            start=False,
                            stop=(ko == KO - 1),
                        )
                    return zp

                # gate order i, g, f, o: each gate's ACT overlaps the next
                # gate's matmuls; o last so h is ready soonest after the MMs.
                g_i = wp.tile([BS, UNITS], f32, tag="gi")
                g_g = wp.tile([BS, UNITS], f32, tag="gg")
                g_f = wp.tile([BS, UNITS], f32, tag="gf")
                g_o = wp.tile([BS, UNITS], f32, tag="go")
                ig = wp.tile([BS, UNITS], f32, tag="ig")
                tc_t = wp.tile([BS, UNITS], f32, tag="tc")
                h = wp.tile([BS, UNITS], f32, tag="h")

                zp = gate_mms(0)
                nc.scalar.activation(g_i, zp, AF.Sigmoid)
                zp = gate_mms(2)
                nc.scalar.activation(g_g, zp, AF.Tanh)
                nc.gpsimd.tensor_mul(out=ig, in0=g_i, in1=g_g)
                zp = gate_mms(1)
                nc.scalar.activation(g_f, zp, AF.Sigmoid)
                nc.gpsimd.tensor_mul(out=cst[nxt], in0=g_f, in1=cst[cur])
                nc.vector.tensor_add(out=cst[nxt], in0=cst[nxt], in1=ig)
                nc.scalar.activation(tc_t, cst[nxt], AF.Tanh)
                zp = gate_mms(3)
                nc.scalar.activation(g_o, zp, AF.Sigmoid)
                # fill PE stall with next step's x transpose
                if xpose_in_ap is not None:
                    tps = xpool.tile([64, BS], f32, tag="xtp")
                    nc.tensor.transpose(tps, xpose_in_ap, ident)
                    nc.vector.tensor_copy(out=xT[:F, xpose_slot, :], in_=tps)
                nc.vector.tensor_mul(out=h, in0=g_o, in1=tc_t)
                # transpose h for next step's stationary
                hps = tpool.tile([128, KO, BS], f32, tag="htp")
                for ko in range(KO):
                    nc.tensor.transpose(
                        hps[:, ko, :], h[:, ko * 128 : (ko + 1) * 128], ident
                    )
                nc.vector.tensor_copy(out=hT[nxt], in_=hps)
                return h

            def warm_body(m, dyn):
                # Steps m*BODY .. m*BODY+31 from static slots; refill each
                # 16-slot half right after its last reader so only the DMA
                # *source* offset is dynamic (ldweights can't take reg APs).
                for k in range(BODY):
                    last_step = (not dyn) and k == BODY - 1
                    if last_step:
                        xp_in, xp_slot = None, None
                    else:
                        xp_slot = (k + 1) % BODY
                        xp_in = xsb[:, xp_slot, :]
                    emit_step(xT[:, k, :], xp_in, xp_slot)
                    if dyn and k == HALF - 1:
                        nc.sync.dma_start(
                            xsb[:, :HALF, :], x_d[:, ds((m + 1) * BODY, HALF), :]
                        )
                    elif dyn and k == BODY - 1:
                        nc.sync.dma_start(
                            xsb[:, HALF:, :],
                            x_d[:, ds((m + 1) * BODY + HALF, HALF), :],
                        )

            def full_round():
                round_prologue()
                if n_bodies > 1:
                    with tc.For_i(
                        0, n_bodies - 1, 1, hint_engines=(mybir.EngineType.PE,)
                    ) as jj:
                        warm_body(jj, dyn=True)
                warm_body(n_bodies - 1, dyn=False)
                forecast()

            # ---- forecast ----
            def emit_pred(s):
                """pred = h @ Wd + bd from hT[state], into preds[:, s, :];
                transpose into pT for the next cell step."""
                cur = state[0]
                pps = xpool.tile([BS, OUT], f32, tag="xtp")
                for ko in range(KO):
                    nc.tensor.matmul(
                        pps,
                        hT[cur][:, ko, :],
                        wd_sb[:, ko, :],
                        start=(ko == 0),
                        stop=False,
                    )
                nc.tensor.matmul(pps, ones_c, wd_sb[:, KO, :], start=False, stop=True)
                nc.vector.tensor_copy(out=preds[:, s, :], in_=pps)
                if s < STEPS - 1:
                    ptp = xpool.tile([64, BS], f32, tag="xtp")
                    nc.tensor.transpose(ptp, preds[:, s, :], ident)
                    nc.vector.tensor_copy(out=pT[s % 2][:F, :], in_=ptp)

            def forecast():
                emit_pred(0)
                for s in range(1, STEPS):
                    emit_step(pT[(s - 1) % 2], None, None)
                    emit_pred(s)

            if rounds > 1:
                with tc.For_i(
                    0, rounds, 1, hint_engines=(mybir.EngineType.PE,)
                ) as _r:
                    full_round()
            else:
                full_round()

            nc.sync.dma_start(out_d[:, :, :], preds)

    nc.finalize()
    return nc


_cache = {}


def _get_nc(Tv, Sv, rounds=1):
    key = (Tv, Sv, rounds)
    if key not in _cache:
        from concourse import bacc

        nc = bacc.Bacc()
        _cache[key] = _build(nc, Tv, Sv, rounds)
    return _cache[key]


def kernel(x, W, U, b, Wd, bd, forecast_steps, _trace=False, _rounds=1):
    x = np.ascontiguousarray(x, dtype=np.float32)
    Tv = x.shape[1]
    Sv = int(forecast_steps)
    assert x.shape[0] == B and x.shape[2] == F

    global T, STEPS
    T, STEPS = Tv, Sv  # allow reduced configs in testing
    nc = _get_nc(Tv, Sv, _rounds)

    shared = {
        "W": np.ascontiguousarray(W, dtype=np.float32),
        "U": np.ascontiguousarray(U, dtype=np.float32),
        "b": np.ascontiguousarray(b, dtype=np.float32),
        "Wd": np.ascontiguousarray(Wd, dtype=np.float32),
        "bd": np.ascontiguousarray(bd, dtype=np.float32),
    }
    in_maps = [
        dict(shared, x=np.ascontiguousarray(x[c * BS : (c + 1) * BS]))
        for c in range(NCORES)
    ]
    from concourse.bass_utils import run_bass_kernel_spmd

    res = run_bass_kernel_spmd(
        nc, in_maps, core_ids=list(range(NCORES)), trace=_trace
    )
    out = np.concatenate([r["out"] for r in res.results], axis=0)
    if _trace:
        return out, res
    return out
